# revision 1
# baseline (speedup 1.0000x reference)
"""DetectionLoss Trainium2 kernel (v2: fp16 pairwise + soft-argmax gather).

Data-parallel over batch: B=16 split across 8 NeuronCores (2 batches/core).
Each core computes masked partial sums (cls_sum, box_sum, obj_sum, count)
over its 2x16x1000 predictions; host combines the 8 partial vectors and does
the final division.

Math notes (vs the jax reference):
- argmax_g iou(p,g) == argmax_g [ln(inter) - ln(ap+ag)] because
  iou = r/(1-r) with r = inter/(ap+ag) monotone increasing in r.
- The argmax one-hot is replaced by a sharp softmax: w = exp(K*(d + ln2))
  with d = ln(inter+eps) - ln(ap+ag) <= -ln2, so w <= 1 and w(iou=0.5) ~ 3e-4
  (normal in fp16). Matched GT attrs = (sum_g w*attr)/(sum_g w); the ones
  column of the gather rhs produces the denominator for free. K=20 makes
  second-best leakage < 2e-4.
- mask = (max_iou > 0.5) is recomputed in stage 2 as iou(pred, matched) > 0.5
  which is the same value, so the pairwise stage never reduces/thresholds.
- cls BCE term collapses: one-hot vs one-hot BCE mean over 80 classes is
  C0 + (pred_cls != matched_cls)/80 elementwise; matched_cls from the soft
  gather is compared with |diff| < 0.5.
- The pairwise stage runs in fp16 (coords <= 640, areas <= 4e4: in range;
  rel err ~5e-4 << 2e-2 budget). Stage 2 (elementwise GIoU etc.) is fp32.
- Gather: d is transposed per-t on the PE (fp16, 1-pass); the Exp activation
  reads the transposed PSUM and writes the fp16 gather operand directly, so
  no separate one-hot tensor or PSUM copy exists.

Prediction index p is remapped p = r*8 + t (r: 125 partitions, t: 8 free
columns); all loss sums are permutation invariant so the remap is free.
"""

import sys

sys.path.insert(0, "/opt/trn_rl_repo")

import numpy as np

B, F, P, G = 16, 16, 1000, 100
NCORES = 8
BL = B // NCORES          # batches per core
BF = BL * F               # (b,f) pairs per core
R, T = 125, 8             # p = r*8 + t
NUM_CLASSES = 80

_LOG2 = 0.6931471805599453
_SP1 = 0.31326168751822286        # log1p(exp(-1))
C0 = (_SP1 + (NUM_CLASSES - 1) * _LOG2) / NUM_CLASSES
C1 = 1.0 / NUM_CLASSES
K_SOFT = 20.0
CK = K_SOFT * _LOG2               # exp(K*d + CK) = (2r)^K <= 1
EPS16 = 6.104e-5                  # fp16 min normal; ln bias

_CACHE = {}


def _build(mode="full"):
    mode_set = set(mode.split(","))
    import concourse.bass as bass
    import concourse.bacc as bacc
    import concourse.tile as tile
    from concourse import mybir
    from concourse.masks import make_identity

    f32 = mybir.dt.float32
    f16 = mybir.dt.float16
    i32 = mybir.dt.int32
    Alu = mybir.AluOpType
    Act = mybir.ActivationFunctionType

    nc = bacc.Bacc(None)
    pb_d = nc.dram_tensor("pb", [BF, R, T, 4], f32, kind="ExternalInput")
    sc_d = nc.dram_tensor("sc", [BF, R, T], f32, kind="ExternalInput")
    pc_d = nc.dram_tensor("pc", [BF, R, T], i32, kind="ExternalInput")
    gtb_d = nc.dram_tensor("gtb", [BF, G, 4], f32, kind="ExternalInput")
    gtc_d = nc.dram_tensor("gtc", [BF, G], i32, kind="ExternalInput")
    out_d = nc.dram_tensor("partials", [4, 1], f32, kind="ExternalOutput")

    with tile.TileContext(nc) as tc:
        with (
            tc.tile_pool(name="st", bufs=1) as st,
            tc.tile_pool(name="pair", bufs=2) as pair,
            tc.tile_pool(name="grp", bufs=5) as grp_pool,
            tc.tile_pool(name="wsb", bufs=3) as wsb,
            tc.tile_pool(name="s2", bufs=1) as s2,
            tc.tile_pool(name="ps_pl", bufs=2, space="PSUM") as ps_pl,
            tc.tile_pool(name="ps_tr", bufs=3, space="PSUM") as ps_tr,
            tc.tile_pool(name="ps_mg", bufs=2, space="PSUM") as ps_mg,
        ):
            # ---- static setup -------------------------------------------
            identh = st.tile([128, 128], f16)
            make_identity(nc, identh[:])
            onesh_row = st.tile([1, 128], f16)
            nc.vector.memset(onesh_row[:], 1.0)
            ones_col = st.tile([128, 1], f32)
            nc.vector.memset(ones_col[:], 1.0)
            zero_b = st.tile([128, 1], f32)
            nc.vector.memset(zero_b[:], 0.0)
            one_b = st.tile([128, 1], f32)
            nc.vector.memset(one_b[:], 1.0)
            epsh_b = st.tile([128, 1], f16)
            nc.vector.memset(epsh_b[:], EPS16)
            zeroh_b = st.tile([128, 1], f16)
            nc.vector.memset(zeroh_b[:], 0.0)
            ck_b = st.tile([128, 1], f32)
            nc.vector.memset(ck_b[:], CK)

            # ---- whole-core input loads ---------------------------------
            pb_all = st.tile([R, BF, T, 4], f32)
            sc_all = st.tile([R, BF, T], f32)
            pc_i = st.tile([R, BF, T], i32)
            nc.sync.dma_start(out=pb_all[:], in_=pb_d.rearrange("a r t c -> r a t c"))
            nc.sync.dma_start(out=sc_all[:], in_=sc_d.rearrange("a r t -> r a t"))
            nc.sync.dma_start(out=pc_i[:], in_=pc_d.rearrange("a r t -> r a t"))
            pc_all = st.tile([R, BF, T], f32)
            nc.vector.tensor_copy(out=pc_all[:], in_=pc_i[:])

            gtb_row = st.tile([BF, G, 4], f32)      # bf on partitions
            nc.sync.dma_start(out=gtb_row[:], in_=gtb_d[:])
            gtb_g = st.tile([G, BF, 4], f32)        # g on partitions
            nc.sync.dma_start(out=gtb_g[:], in_=gtb_d.rearrange("a g c -> g a c"))
            gtc_g_i = st.tile([G, BF], i32)
            nc.sync.dma_start(out=gtc_g_i[:], in_=gtc_d.rearrange("a g -> g a"))

            # ---- pred tables --------------------------------------------
            # predh[..., 0:4] = coords, [..., 4] = area (fp16, pairwise use)
            predh = st.tile([R, BF, T, 6], f16)
            for c in range(4):
                nc.vector.tensor_copy(out=predh[:, :, :, c], in_=pb_all[:, :, :, c])
            wp = st.tile([R, BF, T], f32)
            hp = st.tile([R, BF, T], f32)
            ap_all = st.tile([R, BF, T], f32)
            nc.vector.tensor_tensor(out=wp[:], in0=pb_all[:, :, :, 2], in1=pb_all[:, :, :, 0], op=Alu.subtract)
            nc.vector.tensor_tensor(out=hp[:], in0=pb_all[:, :, :, 3], in1=pb_all[:, :, :, 1], op=Alu.subtract)
            nc.vector.tensor_tensor(out=ap_all[:], in0=wp[:], in1=hp[:], op=Alu.mult)
            nc.vector.tensor_copy(out=predh[:, :, :, 4], in_=ap_all[:])

            # ---- gt tables ----------------------------------------------
            # gt5h_row[bf, c, g]: x1,y1,x2,y2,ag (fp16 rows for planes bcast)
            gt5_row = st.tile([BF, 5, G], f32)
            for c in range(4):
                nc.vector.tensor_copy(out=gt5_row[:, c, :], in_=gtb_row[:, :, c])
            wgr = st.tile([BF, G], f32)
            hgr = st.tile([BF, G], f32)
            nc.vector.tensor_tensor(out=wgr[:], in0=gtb_row[:, :, 2], in1=gtb_row[:, :, 0], op=Alu.subtract)
            nc.vector.tensor_tensor(out=hgr[:], in0=gtb_row[:, :, 3], in1=gtb_row[:, :, 1], op=Alu.subtract)
            nc.vector.tensor_tensor(out=gt5_row[:, 4, :], in0=wgr[:], in1=hgr[:], op=Alu.mult)
            gt5h_row = st.tile([BF, 5, G], f16)
            nc.vector.tensor_copy(out=gt5h_row[:], in_=gt5_row[:])

            # attr7h[g, bf, c]: x1,y1,x2,y2,ag,cls,1 (fp16 gather rhs)
            attr7 = st.tile([G, BF, 7], f32)
            nc.vector.tensor_copy(out=attr7[:, :, 0:4], in_=gtb_g[:])
            wgg = st.tile([G, BF], f32)
            hgg = st.tile([G, BF], f32)
            nc.vector.tensor_tensor(out=wgg[:], in0=gtb_g[:, :, 2], in1=gtb_g[:, :, 0], op=Alu.subtract)
            nc.vector.tensor_tensor(out=hgg[:], in0=gtb_g[:, :, 3], in1=gtb_g[:, :, 1], op=Alu.subtract)
            nc.vector.tensor_tensor(out=attr7[:, :, 4], in0=wgg[:], in1=hgg[:], op=Alu.mult)
            nc.vector.tensor_copy(out=attr7[:, :, 5], in_=gtc_g_i[:])
            nc.vector.memset(attr7[:, :, 6], 1.0)
            attr7h = st.tile([G, BF, 7], f16)
            nc.vector.tensor_copy(out=attr7h[:], in_=attr7[:])

            matched = st.tile([R, BF, T, 7], f32)
            if mode_set & {"nopair", "nogather"}:
                nc.vector.memset(matched[:], 1.0)

            # ---- per-(b,f) pipeline, groups of NB -----------------------
            # Grouping batches the Ln and Exp activations so the activation
            # table only swaps twice per group (ln set <-> exp set) instead
            # of twice per bf.
            NB = 4

            def pd_b(bf, c):
                return predh[:, bf, :, c].unsqueeze(2).broadcast_to([R, T, G])

            sh = [R, T, G]
            for g0 in range(0, BF, NB) if "nopair" not in mode_set else []:
                grp = list(range(g0, min(g0 + NB, BF)))
                gt = {}
                for bf in grp:
                    # fp16 gt row -> partition 0 -> broadcast matmul; the
                    # corner ops read the fp16 PSUM planes directly, only the
                    # area plane is copied to SBUF for the Pool engine.
                    row5 = pair.tile([1, 5, G], f16)
                    nc.sync.dma_start(out=row5[:], in_=gt5h_row[bf : bf + 1, :, :])
                    pl_ps = ps_pl.tile([R, 5, G], f32)
                    nc.tensor.matmul(pl_ps[:], onesh_row[0:1, :R], row5[:])
                    ag_sb = pair.tile([R, 1, G], f16)
                    nc.scalar.copy(ag_sb[:], pl_ps[:, 4:5, :])

                    def pl_b(c):
                        return pl_ps[:, c, :].unsqueeze(1).broadcast_to([R, T, G])

                    t1x = pair.tile(sh, f16)
                    t2x = pair.tile(sh, f16)
                    t1y = pair.tile(sh, f16)
                    t2y = pair.tile(sh, f16)
                    wx = pair.tile(sh, f16)
                    wy = pair.tile(sh, f16)
                    rx = pair.tile(sh, f16)
                    ry = pair.tile(sh, f16)
                    inter = grp_pool.tile(sh, f16)
                    apag = grp_pool.tile(sh, f16)
                    nc.vector.tensor_tensor(out=t1x[:], in0=pl_b(0), in1=pd_b(bf, 0), op=Alu.max)
                    nc.vector.tensor_tensor(out=t2x[:], in0=pl_b(2), in1=pd_b(bf, 2), op=Alu.min)
                    nc.vector.tensor_tensor(out=t1y[:], in0=pl_b(1), in1=pd_b(bf, 1), op=Alu.max)
                    nc.vector.tensor_tensor(out=t2y[:], in0=pl_b(3), in1=pd_b(bf, 3), op=Alu.min)
                    nc.vector.tensor_tensor(out=wx[:], in0=t2x[:], in1=t1x[:], op=Alu.subtract)
                    nc.gpsimd.tensor_tensor(out=wy[:], in0=t2y[:], in1=t1y[:], op=Alu.subtract)
                    nc.vector.tensor_scalar(out=rx[:], in0=wx[:], scalar1=0.0, scalar2=None, op0=Alu.max)
                    nc.vector.tensor_scalar(out=ry[:], in0=wy[:], scalar1=0.0, scalar2=None, op0=Alu.max)
                    nc.vector.tensor_tensor(out=inter[:], in0=rx[:], in1=ry[:], op=Alu.mult)
                    nc.gpsimd.tensor_tensor(
                        out=apag[:],
                        in0=ag_sb[:, 0, :].unsqueeze(1).broadcast_to([R, T, G]),
                        in1=pd_b(bf, 4),
                        op=Alu.add,
                    )
                    gt[bf] = [inter, apag]
                for bf in grp:
                    inter, apag = gt[bf]
                    li = grp_pool.tile(sh, f16)
                    lc = grp_pool.tile(sh, f16)
                    nc.scalar.activation(out=li[:], in_=inter[:], func=Act.Ln, bias=epsh_b[:R], scale=1.0)
                    nc.scalar.activation(out=lc[:], in_=apag[:], func=Act.Ln, bias=zeroh_b[:R], scale=1.0)
                    gt[bf] += [li, lc]
                for bf in grp:
                    li, lc = gt[bf][2], gt[bf][3]
                    d = grp_pool.tile(sh, f16)
                    nc.gpsimd.tensor_tensor(out=d[:], in0=li[:], in1=lc[:], op=Alu.subtract)
                    gt[bf].append(d)
                if "nogather" in mode_set:
                    continue
                for bf in grp:
                    # transpose d per t (fp16, 1-pass); Exp reads the PSUM
                    # and writes the fp16 soft one-hot directly
                    d = gt[bf][4]
                    dt_ps = ps_tr.tile([G, T, 128], f16)
                    for t in range(T):
                        nc.tensor.transpose(dt_ps[:, t, :R], d[:, t, :], identh[:R, :R])
                    w_sb = wsb.tile([G, T, 128], f16)
                    nc.scalar.activation(
                        out=w_sb[:, :, :R], in_=dt_ps[:, :, :R], func=Act.Exp,
                        bias=ck_b[:G], scale=K_SOFT,
                    )
                    gt[bf].append(w_sb)
                for bf in grp:
                    w_sb = gt[bf][5]
                    mg_ps = ps_mg.tile([R, T, 7], f32)
                    for t in range(T):
                        nc.tensor.matmul(mg_ps[:, t, :], w_sb[:, t, :R], attr7h[:, bf, :])
                    nc.scalar.copy(matched[:, bf, :, :], mg_ps[:])

            # ---- stage 2: normalize, GIoU/cls/obj + masked sums ---------
            def pbc(c):
                return pb_all[:, :, :, c]

            sh2 = [R, BF, T]
            den_c = s2.tile(sh2, f32)
            rden = s2.tile(sh2, f32)
            nc.vector.tensor_scalar(out=den_c[:], in0=matched[:, :, :, 6], scalar1=1e-30, scalar2=None, op0=Alu.max)
            nc.vector.reciprocal(out=rden[:], in_=den_c[:])
            mgn = s2.tile([R, BF, T, 6], f32)
            for c in range(3):
                nc.vector.tensor_tensor(out=mgn[:, :, :, c], in0=matched[:, :, :, c], in1=rden[:], op=Alu.mult)
            for c in range(3, 6):
                nc.gpsimd.tensor_tensor(out=mgn[:, :, :, c], in0=matched[:, :, :, c], in1=rden[:], op=Alu.mult)

            def mgc(c):
                return mgn[:, :, :, c]

            ltx = s2.tile(sh2, f32)
            lty = s2.tile(sh2, f32)
            rbx = s2.tile(sh2, f32)
            rby = s2.tile(sh2, f32)
            wx2 = s2.tile(sh2, f32)
            wy2 = s2.tile(sh2, f32)
            rx2 = s2.tile(sh2, f32)
            ry2 = s2.tile(sh2, f32)
            inter2 = s2.tile(sh2, f32)
            u1 = s2.tile(sh2, f32)
            union2 = s2.tile(sh2, f32)
            elx = s2.tile(sh2, f32)
            ely = s2.tile(sh2, f32)
            erx = s2.tile(sh2, f32)
            ery = s2.tile(sh2, f32)
            ew = s2.tile(sh2, f32)
            eh = s2.tile(sh2, f32)
            earea = s2.tile(sh2, f32)
            ru = s2.tile(sh2, f32)
            re_ = s2.tile(sh2, f32)
            iou2 = s2.tile(sh2, f32)
            esu = s2.tile(sh2, f32)
            t3 = s2.tile(sh2, f32)
            b1 = s2.tile(sh2, f32)
            box_per = s2.tile(sh2, f32)
            mask_all = s2.tile(sh2, f32)
            ddc = s2.tile(sh2, f32)
            adc = s2.tile(sh2, f32)
            eqc = s2.tile(sh2, f32)
            cls_per = s2.tile(sh2, f32)
            sabs = s2.tile(sh2, f32)
            sexp = s2.tile(sh2, f32)
            sln = s2.tile(sh2, f32)
            srelu = s2.tile(sh2, f32)
            obj_per = s2.tile(sh2, f32)
            scratch = s2.tile(sh2, f32)
            accs = s2.tile([R, 4], f32)

            nc.vector.tensor_tensor(out=ltx[:], in0=pbc(0), in1=mgc(0), op=Alu.max)
            nc.vector.tensor_tensor(out=lty[:], in0=pbc(1), in1=mgc(1), op=Alu.max)
            nc.vector.tensor_tensor(out=rbx[:], in0=pbc(2), in1=mgc(2), op=Alu.min)
            nc.vector.tensor_tensor(out=rby[:], in0=pbc(3), in1=mgc(3), op=Alu.min)
            nc.gpsimd.tensor_tensor(out=wx2[:], in0=rbx[:], in1=ltx[:], op=Alu.subtract)
            nc.gpsimd.tensor_tensor(out=wy2[:], in0=rby[:], in1=lty[:], op=Alu.subtract)
            nc.vector.tensor_scalar(out=rx2[:], in0=wx2[:], scalar1=0.0, scalar2=None, op0=Alu.max)
            nc.vector.tensor_scalar(out=ry2[:], in0=wy2[:], scalar1=0.0, scalar2=None, op0=Alu.max)
            nc.vector.tensor_tensor(out=inter2[:], in0=rx2[:], in1=ry2[:], op=Alu.mult)
            nc.gpsimd.tensor_tensor(out=u1[:], in0=ap_all[:], in1=mgc(4), op=Alu.add)
            nc.vector.tensor_tensor(out=union2[:], in0=u1[:], in1=inter2[:], op=Alu.subtract)
            nc.vector.tensor_tensor(out=elx[:], in0=pbc(0), in1=mgc(0), op=Alu.min)
            nc.vector.tensor_tensor(out=ely[:], in0=pbc(1), in1=mgc(1), op=Alu.min)
            nc.vector.tensor_tensor(out=erx[:], in0=pbc(2), in1=mgc(2), op=Alu.max)
            nc.vector.tensor_tensor(out=ery[:], in0=pbc(3), in1=mgc(3), op=Alu.max)
            nc.gpsimd.tensor_tensor(out=ew[:], in0=erx[:], in1=elx[:], op=Alu.subtract)
            nc.gpsimd.tensor_tensor(out=eh[:], in0=ery[:], in1=ely[:], op=Alu.subtract)
            nc.vector.tensor_tensor(out=earea[:], in0=ew[:], in1=eh[:], op=Alu.mult)
            nc.vector.reciprocal(out=ru[:], in_=union2[:])
            nc.vector.reciprocal(out=re_[:], in_=earea[:])
            nc.vector.tensor_tensor(out=iou2[:], in0=inter2[:], in1=ru[:], op=Alu.mult)
            nc.gpsimd.tensor_tensor(out=esu[:], in0=earea[:], in1=union2[:], op=Alu.subtract)
            nc.vector.tensor_tensor(out=t3[:], in0=esu[:], in1=re_[:], op=Alu.mult)
            nc.vector.tensor_tensor(out=b1[:], in0=t3[:], in1=iou2[:], op=Alu.subtract)
            nc.vector.tensor_scalar(out=box_per[:], in0=b1[:], scalar1=1.0, scalar2=None, op0=Alu.add)
            nc.vector.tensor_scalar(out=mask_all[:], in0=iou2[:], scalar1=0.5, scalar2=None, op0=Alu.is_gt)
            # cls: matched class is a soft average; accept |diff| < 0.5
            nc.vector.tensor_tensor(out=ddc[:], in0=pc_all[:], in1=mgc(5), op=Alu.subtract)
            nc.scalar.activation(out=adc[:], in_=ddc[:], func=Act.Abs, bias=zero_b[:R], scale=1.0)
            nc.vector.tensor_scalar(out=eqc[:], in0=adc[:], scalar1=0.5, scalar2=None, op0=Alu.is_lt)
            nc.vector.tensor_scalar(
                out=cls_per[:], in0=eqc[:], scalar1=-C1, scalar2=C0 + C1, op0=Alu.mult, op1=Alu.add
            )
            # obj: softplus(-s) = relu(-s) + ln(1 + exp(-|s|))
            nc.scalar.activation(out=sabs[:], in_=sc_all[:], func=Act.Abs, bias=zero_b[:R], scale=1.0)
            nc.scalar.activation(out=sexp[:], in_=sabs[:], func=Act.Exp, bias=zero_b[:R], scale=-1.0)
            nc.scalar.activation(out=sln[:], in_=sexp[:], func=Act.Ln, bias=one_b[:R], scale=1.0)
            nc.scalar.activation(out=srelu[:], in_=sc_all[:], func=Act.Relu, bias=zero_b[:R], scale=-1.0)
            nc.gpsimd.tensor_tensor(out=obj_per[:], in0=sln[:], in1=srelu[:], op=Alu.add)
            # masked sums -> accs columns (plain mult + reduce; accum_out
            # variants of TS/TTR fail at runtime on this stack)
            nc.vector.tensor_tensor(out=scratch[:], in0=cls_per[:], in1=mask_all[:], op=Alu.mult)
            nc.vector.tensor_reduce(out=accs[:, 0:1], in_=scratch[:], axis=mybir.AxisListType.XY, op=Alu.add)
            nc.vector.tensor_tensor(out=box_per[:], in0=box_per[:], in1=mask_all[:], op=Alu.mult)
            nc.vector.tensor_reduce(out=accs[:, 1:2], in_=box_per[:], axis=mybir.AxisListType.XY, op=Alu.add)
            nc.vector.tensor_tensor(out=obj_per[:], in0=obj_per[:], in1=mask_all[:], op=Alu.mult)
            nc.vector.tensor_reduce(out=accs[:, 2:3], in_=obj_per[:], axis=mybir.AxisListType.XY, op=Alu.add)
            nc.vector.tensor_reduce(out=accs[:, 3:4], in_=mask_all[:], axis=mybir.AxisListType.XY, op=Alu.add)
            fin_ps = ps_mg.tile([4, 1], f32, bufs=1)
            nc.tensor.matmul(fin_ps[:], accs[:], ones_col[:R, :])
            fin_sb = s2.tile([4, 1], f32)
            nc.scalar.copy(fin_sb[:], fin_ps[:])
            nc.sync.dma_start(out=out_d[:], in_=fin_sb[:])

    nc.finalize()
    return nc


def _get_nc():
    import os
    mode = os.environ.get("KMODE", "full")
    if "nc" not in _CACHE:
        _CACHE["nc"] = _build(mode)
    return _CACHE["nc"]


def _make_in_maps(pred_boxes, pred_scores, pred_classes, gt_boxes, gt_classes):
    in_maps = []
    for c in range(NCORES):
        sl = slice(c * BL, (c + 1) * BL)
        in_maps.append({
            "pb": np.ascontiguousarray(pred_boxes[sl]).reshape(BF, R, T, 4),
            "sc": np.ascontiguousarray(pred_scores[sl]).reshape(BF, R, T),
            "pc": np.ascontiguousarray(pred_classes[sl]).reshape(BF, R, T),
            "gtb": np.ascontiguousarray(gt_boxes[sl]).reshape(BF, G, 4),
            "gtc": np.ascontiguousarray(gt_classes[sl]).reshape(BF, G),
        })
    return in_maps


def _combine(partials):
    tot = np.zeros(4, dtype=np.float32)
    for p in partials:
        tot += p.reshape(4).astype(np.float32)
    cls_s, box_s, obj_s, n = tot
    denom = np.float32(max(n, 1.0))
    if n > 0:
        cls_l = np.float32(cls_s / denom)
        box_l = np.float32(box_s / denom)
        obj_l = np.float32(obj_s / denom)
    else:
        cls_l = box_l = obj_l = np.float32(0.0)
    loss = np.float32(cls_l + box_l + obj_l)
    return np.stack([loss, cls_l, box_l, obj_l]).astype(np.float32)


def kernel(pred_boxes, pred_scores, pred_classes, gt_boxes, gt_classes):
    from concourse.bass_utils import run_bass_kernel_spmd

    nc = _get_nc()
    in_maps = _make_in_maps(pred_boxes, pred_scores, pred_classes, gt_boxes, gt_classes)
    res = run_bass_kernel_spmd(nc, in_maps, list(range(NCORES)))
    return _combine([res.results[c]["partials"] for c in range(NCORES)])

